# revision 1
# baseline (speedup 1.0000x reference)
"""Trainium2 Bass kernel for nn_AttentionBlock (B=4, C=256, H=W=64, RD=32).

Sharding: 8 cores = (batch b, query-half h). Each core computes the full
attention output for its 2048 queries of one batch element. No collectives.

Math (per core, b fixed, i in its half, j over all 4096 positions):
  q = Wq x + bq            [32, Ni]
  k = Wk x + bk            [32, N]
  vT_aug[j, c'] = (Wv x + bv).T with an extra all-ones column c'=256
  P[j, i]  = exp(k[:,j] . q[:,i])     (unnormalized; |energy| <~ 29 here,
                                       so no max-subtraction is needed)
  outa[c', i] = sum_j vT_aug[j, c'] * P[j, i]  -> rows 0..255 numerator,
                                                  row 256 = Z (denominator)
  out[c, i] = outa[c, i] * (gamma / Z[i]) + x[c, i]

Implementation notes:
  - No transposes anywhere: energy is computed directly in [j, i] layout
    (lhsT = k slice, rhs = q slice); the denominator comes from the ones
    column of vT_aug; gamma/Z is partition-broadcast with a K=1 matmul.
  - All heavy matmuls in float32r (~1.5e-4 relative error).
  - The PE runs throttled at ~1.2 GHz in this environment, so the design
    minimizes total streamed columns: the RD=32 energy matmuls are packed
    4x into the PE array row strips (tile_position), with k/q replicated
    across the four 32-partition strips.
  - The vT bias (along the free dim) is applied by broadcasting [bv,1,0]
    to all partitions once with a K=1 matmul and a DVE add, which also
    plants the ones column used for Z.
  - gamma/Z is partition-broadcast on the otherwise-idle GPSIMD engine.

Measured: ~149-200 us per invocation across all 8 cores (hardware-loop
slope method; best 149 us, varies with the chip's throttle state),
relative error 3.5e-4 vs the fp32 reference. Two buffering wins were
validated by drift-controlled interleaved A/B: a 5-deep energy-PSUM
pipeline (phase-1 projections borrow the output-accumulator banks,
idle during phase 1; ~12% vs 4-deep) and deeper SBUF staging
(P pool 14, output staging 4; ~3%).

build_nc(z_split=...) is an experimental variant (part of Z summed on
the VectorE); it faulted the device at runtime and is OFF (z_split=0)
in the deliverable path.
"""

import contextlib
import os
import sys

for _p in ("/opt/trn_rl_repo", "/root/.axon_site/_ro/trn_rl_repo"):
    if os.path.isdir(_p) and _p not in sys.path:
        sys.path.insert(0, _p)

import numpy as np

import concourse.mybir as mybir
import concourse.tile as tile
from concourse import bacc
from concourse.bass_utils import run_bass_kernel_spmd

B, C, H, W = 4, 256, 64, 64
N = H * W            # 4096 positions
RD = C // 8          # 32 reduced dim
NCORES = 8
NI = N // 2          # 2048 queries per core
GW = 512             # i-group width (PSUM bank = 512 fp32)
G = NI // GW         # 4 i-groups
JB = N // 128        # 32 j-blocks
CA = C + 2           # 258: padded vT columns (256 ch + ones col + pad)

f32 = mybir.dt.float32
f32r = mybir.dt.float32r
Exp = mybir.ActivationFunctionType.Exp
Ident = mybir.ActivationFunctionType.Identity


def build_nc(n_iter: int = 1, pack_energy: bool = True, z_split: int = 0,
             deep_e: bool = True, deep_sb: bool = True, deep2: bool = False,
             early_refill: bool = False):
    nc = bacc.Bacc()

    xr = nc.dram_tensor("xr", [C, N], f32r, kind="ExternalInput")
    xq = nc.dram_tensor("xq", [C, NI], f32r, kind="ExternalInput")
    xfh = nc.dram_tensor("xfh", [C, NI], f32, kind="ExternalInput")
    wqt = nc.dram_tensor("wqt", [C, RD], f32r, kind="ExternalInput")
    wkt = nc.dram_tensor("wkt", [C, RD], f32r, kind="ExternalInput")
    wvt = nc.dram_tensor("wvt", [C, CA], f32r, kind="ExternalInput")
    bq_t = nc.dram_tensor("bq", [RD, 1], f32, kind="ExternalInput")
    bk_t = nc.dram_tensor("bk4", [128, 1], f32, kind="ExternalInput")
    bvz_t = nc.dram_tensor("bvz", [1, CA], f32r, kind="ExternalInput")
    one_t = nc.dram_tensor("one_r", [1, 128], f32r, kind="ExternalInput")
    onec_t = nc.dram_tensor("onec", [128, 2], f32r, kind="ExternalInput")
    gamma_t = nc.dram_tensor("gamma", [1, 1], f32, kind="ExternalInput")
    out_t = nc.dram_tensor("out", [C, NI], f32, kind="ExternalOutput")

    with tile.TileContext(nc) as tc:
        with tc.tile_pool(name="const", bufs=1) as cp, \
             tc.tile_pool(name="vtp", bufs=1) as vtp, \
             tc.tile_pool(name="qk", bufs=1) as qkp, \
             tc.tile_pool(name="pp", bufs=(16 if deep2 else (14 if deep_sb else 12))) as pp, \
             tc.tile_pool(name="fin", bufs=(3 if deep2 else 2)) as fp, \
             tc.tile_pool(name="outp", bufs=(4 if deep_sb else 3)) as op_, \
             tc.tile_pool(name="ps_e", bufs=(5 if deep_e else 4),
                          space="PSUM") as ps_e, \
             tc.tile_pool(name="ps_v", bufs=1,
                          space="PSUM" if not deep_e else "SBUF") as ps_v, \
             tc.tile_pool(name="ps_o", bufs=1, space="PSUM") as ps_o:

            # ---- constant loads -------------------------------------------
            xr_sb = [cp.tile([128, N], f32r, tag=f"xr{m}", name=f"xr{m}")
                     for m in range(2)]
            xq_sb = [cp.tile([128, NI], f32r, tag=f"xq{m}", name=f"xq{m}")
                     for m in range(2)]
            xf_sb = [cp.tile([128, NI], f32, tag=f"xf{m}", name=f"xf{m}")
                     for m in range(2)]
            for m in range(2):
                ms = slice(m * 128, (m + 1) * 128)
                nc.sync.dma_start(out=xr_sb[m], in_=xr[ms, :])
                nc.sync.dma_start(out=xq_sb[m], in_=xq[ms, :])
                nc.sync.dma_start(out=xf_sb[m], in_=xfh[ms, :])
            wqt_sb = [cp.tile([128, RD], f32r, tag=f"wqt{m}", name=f"wqt{m}")
                      for m in range(2)]
            wkt_sb = [cp.tile([128, RD], f32r, tag=f"wkt{m}", name=f"wkt{m}")
                      for m in range(2)]
            wvt_sb = [cp.tile([128, CA], f32r, tag=f"wvt{m}", name=f"wvt{m}")
                      for m in range(2)]
            for m in range(2):
                ms = slice(m * 128, (m + 1) * 128)
                nc.sync.dma_start(out=wqt_sb[m], in_=wqt[ms, :])
                nc.sync.dma_start(out=wkt_sb[m], in_=wkt[ms, :])
                nc.sync.dma_start(out=wvt_sb[m], in_=wvt[ms, :])
            bq_sb = cp.tile([RD, 1], f32, tag="bq", name="bq_sb")
            nc.sync.dma_start(out=bq_sb, in_=bq_t[:])
            bk_sb = cp.tile([128, 1], f32, tag="bk", name="bk_sb")
            nc.sync.dma_start(out=bk_sb, in_=bk_t[:])
            bvz_sb = cp.tile([1, CA], f32r, tag="bvz", name="bvz_sb")
            nc.sync.dma_start(out=bvz_sb, in_=bvz_t[:])
            one_sb = cp.tile([1, 128], f32r, tag="one", name="one_sb")
            nc.sync.dma_start(out=one_sb, in_=one_t[:])
            onec_sb = cp.tile([128, 2], f32r, tag="onec", name="onec_sb")
            nc.sync.dma_start(out=onec_sb, in_=onec_t[:])
            gamma_sb = cp.tile([1, 1], f32, tag="gamma", name="gamma_sb")
            nc.sync.dma_start(out=gamma_sb, in_=gamma_t[:])

            # [bv, 1, 0] broadcast to all 128 partitions (also plants the
            # ones column used for the softmax denominator)
            pbv = ps_e.tile([128, CA], f32, tag="pe", name="pbv")
            nc.tensor.matmul(pbv, one_sb, bvz_sb, start=True, stop=True)
            bvbc_sb = cp.tile([128, CA], f32, tag="bvbc", name="bvbc_sb")
            nc.vector.tensor_copy(bvbc_sb, pbv)

            # persistent activation tiles; k/q replicated across the four
            # 32-partition row strips for packed energy matmuls
            vt = [vtp.tile([128, CA], f32r, tag=f"vt{jb}", name=f"vt{jb}")
                  for jb in range(JB)]
            nrep = 4 if pack_energy else 1
            q4 = qkp.tile([32 * nrep, NI], f32r, tag="q", name="q4")
            k4 = qkp.tile([32 * nrep, N], f32r, tag="k", name="k4")

            loop_cm = (tc.For_i(0, n_iter, 1) if n_iter > 1
                       else contextlib.nullcontext())
            with loop_cm:
                # ---- phase 1: projections ---------------------------------
                # q projection into strip 0 (bias per-partition via ACT)
                for g in range(G):
                    gs = slice(g * GW, (g + 1) * GW)
                    pq = (ps_o.tile([RD, GW], f32, tag="o0", name="pq") if deep_e
                          else ps_v.tile([RD, GW], f32, tag="pv", name="pq"))
                    nc.tensor.matmul(pq, wqt_sb[0], xq_sb[0][:, gs],
                                     start=True, stop=False)
                    nc.tensor.matmul(pq, wqt_sb[1], xq_sb[1][:, gs],
                                     start=False, stop=True)
                    nc.scalar.activation(q4[0:RD, gs], pq, Ident, bias=bq_sb)

                # k projection into strip 0
                for g in range(N // GW):
                    gs = slice(g * GW, (g + 1) * GW)
                    pk = (ps_o.tile([RD, GW], f32, tag="o1", name="pk") if deep_e
                          else ps_v.tile([RD, GW], f32, tag="pv", name="pk"))
                    nc.tensor.matmul(pk, wkt_sb[0], xr_sb[0][:, gs],
                                     start=True, stop=False)
                    nc.tensor.matmul(pk, wkt_sb[1], xr_sb[1][:, gs],
                                     start=False, stop=True)
                    nc.scalar.activation(k4[0:RD, gs], pk, Ident,
                                         bias=bk_sb[0:RD, :])

                # replicate q/k to the other strips; these DMAs hide under
                # the vT matmuls below
                for t in range(1, nrep):
                    ts_ = slice(32 * t, 32 * (t + 1))
                    nc.sync.dma_start(out=q4[ts_, :], in_=q4[0:RD, :])
                    nc.sync.dma_start(out=k4[ts_, :], in_=k4[0:RD, :])

                # ---- phase 2 pipeline (energy+exp), defined early so the
                # first quads can overlap the vT projection below ----------
                jc_order = list(range(JB))
                eq = [(g, jc) for g in range(G) for jc in jc_order]
                p_tiles = {}
                next_e = 0

                def emit_energy_quad():
                    """Emit a quad (or single) of energy matmuls + exps."""
                    nonlocal next_e
                    for _ in range(nrep):
                        if next_e >= len(eq):
                            return
                        g, jc = eq[next_e]
                        next_e += 1
                        t = (jc % 4) if pack_energy else 0
                        gs = slice(g * GW, (g + 1) * GW)
                        js = slice(jc * 128, (jc + 1) * 128)
                        ts_ = slice(32 * t, 32 * (t + 1))
                        pe = ps_e.tile([128, GW], f32, tag="pe", name="pe")
                        nc.tensor.matmul(
                            pe, k4[ts_, js], q4[ts_, gs],
                            start=True, stop=True,
                            tile_position=(32 * t, 0) if pack_energy else None)
                        pt = pp.tile([128, GW], f32r, tag="P", name="pt")
                        nc.scalar.activation(pt, pe, Exp)
                        p_tiles[(g, jc)] = pt

                # vT_aug j-blocks: x.T @ WvT (+ broadcast [bv,1,0] via DVE);
                # the q/k replication DMAs hide under these matmuls
                for jb in range(JB):
                    js = slice(jb * 128, (jb + 1) * 128)
                    pv = (ps_o.tile([128, CA], f32, tag="o0", name="pv") if deep_e
                          else ps_v.tile([128, CA], f32, tag="pv", name="pv"))
                    nc.tensor.matmul(pv, xr_sb[0][:, js], wvt_sb[0],
                                     start=True, stop=False)
                    nc.tensor.matmul(pv, xr_sb[1][:, js], wvt_sb[1],
                                     start=False, stop=True)
                    nc.vector.tensor_add(vt[jb], pv, bvbc_sb)

                # ---- phase 2: attention -----------------------------------
                emit_energy_quad()
                emit_energy_quad()
                for g in range(G):
                    gs = slice(g * GW, (g + 1) * GW)
                    po = [ps_o.tile([128, GW], f32, tag="o0", name="po0"),
                          ps_o.tile([128, GW], f32, tag="o1", name="po1"),
                          ps_o.tile([2, GW], f32, tag="oz", name="poz")]
                    if z_split:
                        s_ping = fp.tile([128, GW], f32, tag="Sa",
                                         name="s_ping")
                        s_pong = fp.tile([128, GW], f32, tag="Sb",
                                         name="s_pong")
                        s_r = fp.tile([128, GW], f32r, tag="Sr", name="s_r")
                        s_cur = None
                    for oi, jc in enumerate(jc_order):
                        pt = p_tiles.pop((g, jc))
                        first, last = oi == 0, oi == JB - 1
                        nc.tensor.matmul(po[0], vt[jc][:, 0:128], pt,
                                         start=first, stop=last)
                        nc.tensor.matmul(po[1], vt[jc][:, 128:256], pt,
                                         start=first, stop=last)
                        if oi < z_split:
                            # Z contribution summed on DVE into S
                            if oi == 0:
                                nc.vector.tensor_copy(s_ping,
                                                      pt.bitcast(f32))
                                s_cur = s_ping
                            else:
                                s_nxt = (s_pong if s_cur is s_ping
                                         else s_ping)
                                nc.vector.tensor_add(s_nxt, s_cur,
                                                     pt.bitcast(f32))
                                s_cur = s_nxt
                                if oi == z_split - 1:
                                    nc.vector.tensor_copy(s_r, s_cur)
                        else:
                            # Z contribution accumulated on PE
                            nc.tensor.matmul(po[2], vt[jc][:, 256:258], pt,
                                             start=(oi == z_split),
                                             stop=(last and not z_split))
                        if oi % nrep == (1 if early_refill
                                          else nrep - 1):
                            emit_energy_quad()

                    if z_split:
                        # fold colsum(S) into the same oz accumulation
                        nc.tensor.matmul(po[2], onec_sb, s_r,
                                         start=False, stop=True)

                    # gamma / Z broadcast to 128 partitions via K=1 matmul
                    zr = fp.tile([1, GW], f32, tag="zr", name="zr")
                    zt = fp.tile([1, GW], f32, tag="zt", name="zt")
                    nc.vector.reciprocal(zt, po[2][0:1, :])
                    nc.vector.tensor_scalar_mul(zr, zt, gamma_sb)
                    bc = fp.tile([128, GW], f32, tag="bc", name="bc")
                    nc.gpsimd.partition_broadcast(bc, zr)

                    for m in range(2):
                        ot = op_.tile([128, GW], f32, tag=f"ot{m}",
                                      name=f"ot{m}")
                        nc.vector.tensor_mul(ot, po[m], bc)
                        nc.vector.tensor_add(ot, ot, xf_sb[m][:, gs])
                        nc.sync.dma_start(
                            out=out_t[m * 128:(m + 1) * 128, gs], in_=ot)
    nc.finalize()
    return nc


_CACHE = {}


def _get_nc(n_iter: int = 1):
    if n_iter not in _CACHE:
        _CACHE[n_iter] = build_nc(n_iter)
    return _CACHE[n_iter]


def make_in_maps(x, Wq, bq, Wk, bk, Wv, bv, gamma):
    x = np.asarray(x, dtype=np.float32)
    Wq = np.asarray(Wq, dtype=np.float32)
    bq = np.asarray(bq, dtype=np.float32)
    Wk = np.asarray(Wk, dtype=np.float32)
    bk = np.asarray(bk, dtype=np.float32)
    Wv = np.asarray(Wv, dtype=np.float32)
    bv = np.asarray(bv, dtype=np.float32)
    gamma = np.asarray(gamma, dtype=np.float32)

    wqt = np.ascontiguousarray(Wq.T)                  # [C, RD]
    wkt = np.ascontiguousarray(Wk.T)                  # [C, RD]
    wvt = np.zeros((C, CA), dtype=np.float32)         # [Wv.T | 0 | 0]
    wvt[:, :C] = Wv.T
    bvz = np.zeros((1, CA), dtype=np.float32)         # [bv, 1, 0]
    bvz[0, :C] = bv
    bvz[0, C] = 1.0
    one_r = np.ones((1, 128), dtype=np.float32)
    onec = np.ones((128, 2), dtype=np.float32)
    bq2 = bq.reshape(RD, 1)
    bk2 = np.tile(bk.reshape(RD, 1), (4, 1))
    g2 = gamma.reshape(1, 1)

    in_maps = []
    for c in range(NCORES):
        b, half = divmod(c, 2)
        xb = np.ascontiguousarray(x[b].reshape(C, N))
        xh = np.ascontiguousarray(xb[:, half * NI:(half + 1) * NI])
        in_maps.append({
            "xr": xb, "xq": xh, "xfh": xh,
            "wqt": wqt, "wkt": wkt, "wvt": wvt,
            "bq": bq2, "bk4": bk2, "bvz": bvz, "one_r": one_r,
            "onec": onec, "gamma": g2,
        })
    return in_maps


def assemble(results):
    out = np.empty((B, C, N), dtype=np.float32)
    for c in range(NCORES):
        b, half = divmod(c, 2)
        out[b][:, half * NI:(half + 1) * NI] = results[c]["out"]
    return out.reshape(B, C, H, W)


def kernel(x, Wq, bq, Wk, bk, Wv, bv, gamma):
    nc = _get_nc(1)
    in_maps = make_in_maps(x, Wq, bq, Wk, bk, Wv, bv, gamma)
    res = run_bass_kernel_spmd(nc, in_maps, list(range(NCORES)))
    return assemble(res.results)



# revision 2
# speedup vs baseline: 1.6378x; 1.6378x over previous
"""Trainium2 Bass kernel for nn_AttentionBlock (B=4, C=256, H=W=64, RD=32).

Sharding: 8 cores = (batch b, query-half h); each core computes the full
attention output for its 2048 queries, TRANSPOSED (out.T[i, c]), and the
host un-transposes in assemble() for free.

Math (per core, b fixed, i in its half, j over all 4096 positions):
  q = Wq x + bq                       [32, Ni]   (bf16 in, f32 PSUM)
  k = Wk x + bk                       [32, N]
  vT[j, c'] = (Wv x).T, with column c'=256 equal to 1/gamma (persistent)
  P[j, i] = exp(k[:,j] . q[:,i])      (f32 PSUM energies, bf16 P in SBUF)
  acc[i, c'] = sum_j P[j, i] * vT[j, c']   (PE, transposed output)
      c' 0..255 = numerator (without bv), c' 256 = Z/gamma
  out.T[i, c] = acc[i, c] / acc[i, 256] + (x.T[i, c] + gamma*bv[c])
      (the attention bias bv contributes gamma*bv[c] per channel exactly,
       so it is folded into the host-prepared residual)

Design notes (vs the 512-wide-output v1):
  - attn@v streams 16x32x258 = 132K columns instead of 196K: the output
    free dim is the 258-wide channel axis, the softmax denominator lands
    per-partition (per-query), so normalization is a per-partition
    reciprocal + one fused DVE scalar_tensor_tensor (mult, add).
  - The whole energy path (x, Wq/Wk/Wv, q, k, P, vT) is bf16: the PE
    streams bf16 at the same 1 col/cycle as f32r, but LDWEIGHTS gets the
    compiler-automatic Fast Weight Load (2x) only for non-fp32 dtypes.
    Energies/accumulators stay f32 in PSUM. rel err ~6e-3 (gate 2e-2).
  - Energy matmuls packed 4x into 32-row PE tiles (k/q replicated across
    strips); exps are 1024-wide (2 j-blocks per ACT instruction); the
    energy->exp pipeline is primed 4 pairs deep and refilled 2 pairs per
    4 consumed j-blocks (cad_ph=1) -- lookahead shallower than ~4 pairs
    stalls the PE on exp latency (the original v2 bug, +40us).
  - PSUM: 4 banks energy pipeline (2 x [128,1024]) + 4 accumulator banks
    [128,258], borrowed by the phase-1 projections.
  - PSUM evacuation is split across engines (GPSIMD cannot read PSUM):
    q bias on ACT, k bias on DVE, vT copies alternate DVE/ACT, finalize
    alternates DVE-STT / ACT-scale+GPSIMD-add.
  - Z column of vT is a constant (1/gamma): written once outside the
    hardware loop, so vT evacuation is a pure 256-wide copy.

Measured (slope method, interleaved vs v1): ~114-117us median vs v1's
~201us in the same windows; rel err 5.9e-3. Absolute time swings with
the chip's HAM throttle state (best observed windows ~52us).
"""

import contextlib
import os
import sys

for _p in ("/opt/trn_rl_repo", "/root/.axon_site/_ro/trn_rl_repo"):
    if os.path.isdir(_p) and _p not in sys.path:
        sys.path.insert(0, _p)

import numpy as np
import ml_dtypes

import concourse.mybir as mybir
import concourse.tile as tile
from concourse import bacc
from concourse.bass_utils import run_bass_kernel_spmd

B, C, H, W = 4, 256, 64, 64
N = H * W            # 4096 positions
RD = C // 8          # 32 reduced dim
NCORES = 8
NI = N // 2          # 2048 queries per core
GW = 512             # i-group width (PSUM bank = 512 fp32)
G = NI // GW         # 4 i-groups
JB = N // 128        # 32 j-blocks
CA = C + 2           # 258: padded vT columns (256 ch + Z col + pad)

f32 = mybir.dt.float32
f32r = mybir.dt.float32r
bf16 = mybir.dt.bfloat16
Exp = mybir.ActivationFunctionType.Exp
Ident = mybir.ActivationFunctionType.Identity


def build_nc(n_iter: int = 1, pp_bufs: int = 8, prime: int = 4,
             exp_w: int = 2, p_dt=bf16, cad_n: int = 4, cad_ph: int = 1,
             cad_k: int = 2, skip_p1: bool = False, skip_energy: bool = False,
             skip_exp: bool = False, skip_attn: bool = False,
             skip_fin: bool = False, qk_dt=bf16, evac_split: bool = True,
             fin_split: bool = True, outp_bufs: int = 8):
    nc = bacc.Bacc()

    xr = nc.dram_tensor("xr", [C, N], qk_dt, kind="ExternalInput")
    xth = nc.dram_tensor("xth", [NI, C], f32, kind="ExternalInput")
    wqt = nc.dram_tensor("wqt", [C, RD], qk_dt, kind="ExternalInput")
    wkt = nc.dram_tensor("wkt", [C, RD], qk_dt, kind="ExternalInput")
    wvt = nc.dram_tensor("wvt", [C, CA], qk_dt, kind="ExternalInput")
    bq_t = nc.dram_tensor("bq", [RD, 1], f32, kind="ExternalInput")
    bk_t = nc.dram_tensor("bk", [RD, 1], f32, kind="ExternalInput")
    bvz_t = nc.dram_tensor("bvz", [1, CA], f32r, kind="ExternalInput")
    one_t = nc.dram_tensor("one_r", [1, 128], f32r, kind="ExternalInput")
    out_t = nc.dram_tensor("out", [NI, C], f32, kind="ExternalOutput")

    with tile.TileContext(nc) as tc:
        with tc.tile_pool(name="const", bufs=1) as cp, \
             tc.tile_pool(name="vtp", bufs=1) as vtp, \
             tc.tile_pool(name="qk", bufs=1) as qkp, \
             tc.tile_pool(name="pp", bufs=pp_bufs) as pp, \
             tc.tile_pool(name="fin", bufs=2) as fp, \
             tc.tile_pool(name="outp", bufs=outp_bufs) as op_, \
             tc.tile_pool(name="ps_e", bufs=2, space="PSUM") as ps_e, \
             tc.tile_pool(name="ps_a", bufs=1, space="PSUM") as ps_a:

            # ---- constant loads -------------------------------------------
            xr_sb = [cp.tile([128, N], qk_dt, tag=f"xr{m}", name=f"xr{m}")
                     for m in range(2)]
            for m in range(2):
                nc.sync.dma_start(out=xr_sb[m],
                                  in_=xr[m * 128:(m + 1) * 128, :])
            # x.T chunks for the residual: chunk ic at columns ic*C
            xth_sb = cp.tile([128, (NI // 128) * C], f32, tag="xth",
                             name="xth_sb")
            for ic in range(NI // 128):
                nc.sync.dma_start(
                    out=xth_sb[:, ic * C:(ic + 1) * C],
                    in_=xth[ic * 128:(ic + 1) * 128, :])
            wqt_sb = [cp.tile([128, RD], qk_dt, tag=f"wqt{m}", name=f"wqt{m}")
                      for m in range(2)]
            wkt_sb = [cp.tile([128, RD], qk_dt, tag=f"wkt{m}", name=f"wkt{m}")
                      for m in range(2)]
            wvt_sb = [cp.tile([128, CA], qk_dt, tag=f"wvt{m}", name=f"wvt{m}")
                      for m in range(2)]
            for m in range(2):
                ms = slice(m * 128, (m + 1) * 128)
                nc.sync.dma_start(out=wqt_sb[m], in_=wqt[ms, :])
                nc.sync.dma_start(out=wkt_sb[m], in_=wkt[ms, :])
                nc.sync.dma_start(out=wvt_sb[m], in_=wvt[ms, :])
            bq_sb = cp.tile([RD, 1], f32, tag="bq", name="bq_sb")
            nc.sync.dma_start(out=bq_sb, in_=bq_t[:])
            bk_sb = cp.tile([RD, 1], f32, tag="bk", name="bk_sb")
            nc.sync.dma_start(out=bk_sb, in_=bk_t[:])
            bvz_sb = cp.tile([1, CA], f32r, tag="bvz", name="bvz_sb")
            nc.sync.dma_start(out=bvz_sb, in_=bvz_t[:])
            one_sb = cp.tile([1, 128], f32r, tag="one", name="one_sb")
            nc.sync.dma_start(out=one_sb, in_=one_t[:])

            # [bv, 1/gamma, 0] broadcast to all 128 partitions (plants the
            # Z column that folds the gamma multiply into the reciprocal)
            pbv = ps_a.tile([128, CA], f32, tag="a0", name="pbv")
            nc.tensor.matmul(pbv, one_sb, bvz_sb, start=True, stop=True)
            bvbc_sb = cp.tile([128, CA], f32, tag="bvbc", name="bvbc_sb")
            nc.vector.tensor_copy(bvbc_sb, pbv)

            # persistent activation tiles; k/q replicated across the four
            # 32-partition strips for packed energy matmuls
            vt = [vtp.tile([128, CA], p_dt, tag=f"vt{jb}", name=f"vt{jb}")
                  for jb in range(JB)]
            for jb in range(JB):
                nc.vector.tensor_copy(vt[jb][:, C:CA], bvbc_sb[:, C:CA])
            q4 = qkp.tile([128, NI], qk_dt, tag="q", name="q4")
            k4 = qkp.tile([128, N], qk_dt, tag="k", name="k4")

            if skip_p1:
                nc.sync.dma_start(out=q4, in_=xr[0:128, 0:NI])
                nc.sync.dma_start(out=k4, in_=xr[0:128, :])
                for jb in range(JB):
                    nc.vector.tensor_copy(vt[jb][:, 0:C], bvbc_sb[:, 0:C])

            # this core's query half: column offset into xr (set per-core
            # via the input map by passing xr pre-rolled; see make_in_maps)
            loop_cm = (tc.For_i(0, n_iter, 1) if n_iter > 1
                       else contextlib.nullcontext())
            with loop_cm:
                P1 = not skip_p1
                # ---- phase 1: projections ---------------------------------
                # q projection into strip 0 (queries are xr columns 0:NI
                # after the host roll) with per-partition bias via ACT
                for g in range(G if P1 else 0):
                    gs = slice(g * GW, (g + 1) * GW)
                    pq = ps_a.tile([RD, GW], f32, tag=f"a{g % 4}", name="pq")
                    nc.tensor.matmul(pq, wqt_sb[0], xr_sb[0][:, gs],
                                     start=True, stop=False)
                    nc.tensor.matmul(pq, wqt_sb[1], xr_sb[1][:, gs],
                                     start=False, stop=True)
                    nc.scalar.activation(q4[0:RD, gs], pq, Ident, bias=bq_sb)

                # k projection into strip 0
                for g in range(N // GW if P1 else 0):
                    gs = slice(g * GW, (g + 1) * GW)
                    pk = ps_a.tile([RD, GW], f32, tag=f"a{g % 4}", name="pk")
                    nc.tensor.matmul(pk, wkt_sb[0], xr_sb[0][:, gs],
                                     start=True, stop=False)
                    nc.tensor.matmul(pk, wkt_sb[1], xr_sb[1][:, gs],
                                     start=False, stop=True)
                    if evac_split:
                        nc.vector.tensor_scalar_add(k4[0:RD, gs], pk, bk_sb)
                    else:
                        nc.scalar.activation(k4[0:RD, gs], pk, Ident,
                                             bias=bk_sb)

                # replicate q/k to strips 1-3; these DMAs hide under the vT
                # matmuls below
                for t in range(1, 4 if P1 else 1):
                    ts_ = slice(32 * t, 32 * (t + 1))
                    nc.sync.dma_start(out=q4[ts_, :], in_=q4[0:RD, :])
                    nc.sync.dma_start(out=k4[ts_, :], in_=k4[0:RD, :])

                # vT_aug j-blocks: x.T @ WvT (+ broadcast [bv,1/gamma,0])
                for jb in range(JB if P1 else 0):
                    js = slice(jb * 128, (jb + 1) * 128)
                    pv = ps_a.tile([128, CA], f32, tag=f"a{jb % 4}",
                                   name="pv")
                    nc.tensor.matmul(pv, xr_sb[0][:, js], wvt_sb[0],
                                     start=True, stop=False)
                    nc.tensor.matmul(pv, xr_sb[1][:, js], wvt_sb[1],
                                     start=False, stop=True)
                    if evac_split and jb % 2 == 1:
                        nc.scalar.activation(vt[jb][:, 0:C], pv[:, 0:C],
                                             Ident)
                    else:
                        nc.vector.tensor_copy(vt[jb][:, 0:C], pv[:, 0:C])

                # ---- energy + exp pipeline --------------------------------
                # pair = 2 packed energy matmuls (strips 2m, 2m+1) into one
                # 2-bank PSUM tile, drained by a single 1024-wide exp
                EW = exp_w * GW
                eq = [(g, jp) for g in range(G) for jp in range(JB // exp_w)]
                p_tiles = {}
                next_e = 0

                def emit_energy_pair():
                    nonlocal next_e
                    if next_e >= len(eq):
                        return
                    g, jp = eq[next_e]
                    next_e += 1
                    gs = slice(g * GW, (g + 1) * GW)
                    pe2 = ps_e.tile([128, EW], f32, tag="pe", name="pe2")
                    ew = 8 if skip_energy else GW
                    for h in range(exp_w):
                        jc = jp * exp_w + h
                        t = jc % 4
                        js = slice(jc * 128, (jc + 1) * 128)
                        ts_ = slice(32 * t, 32 * (t + 1))
                        nc.tensor.matmul(
                            pe2[:, h * GW:h * GW + ew],
                            k4[ts_, js], q4[ts_, gs][:, 0:ew],
                            start=True, stop=True,
                            tile_position=(32 * t, 0))
                    pt2 = pp.tile([128, EW], p_dt, tag="P", name="pt2")
                    if skip_exp:
                        nc.scalar.activation(pt2[:, 0:8], pe2[:, 0:8], Exp)
                    else:
                        nc.scalar.activation(pt2, pe2, Exp)
                    p_tiles[(g, jp)] = pt2

                for _ in range(prime):
                    emit_energy_pair()

                # ---- phase 2: attention (transposed output) ---------------
                for g in range(G):
                    ac = [ps_a.tile([128, CA], f32, tag=f"a{t}",
                                    name=f"ac{t}") for t in range(4)]
                    for jc in range(JB):
                        jp, h = divmod(jc, exp_w)
                        pt2 = p_tiles[(g, jp)]
                        first, last = jc == 0, jc == JB - 1
                        for t in range(0 if skip_attn else 4):
                            lo = h * GW + t * 128
                            nc.tensor.matmul(ac[t], pt2[:, lo:lo + 128],
                                             vt[jc], start=first, stop=last)
                        if h == exp_w - 1:
                            p_tiles.pop((g, jp))
                        if jc % cad_n == cad_ph:
                            for _ in range(cad_k):
                                emit_energy_pair()

                    # finalize: per-partition gamma/Z scale + residual
                    for t in range(0 if skip_fin else 4):
                        ic = g * 4 + t
                        zr = fp.tile([128, 1], f32, tag="zr", name="zr")
                        nc.vector.reciprocal(zr, ac[t][:, 256:257])
                        ot = op_.tile([128, C], f32, tag="ot", name="ot")
                        if fin_split and t % 2 == 1:
                            nc.scalar.activation(ot, ac[t][:, 0:C], Ident,
                                                 scale=zr)
                            nc.gpsimd.tensor_add(
                                ot, ot, xth_sb[:, ic * C:(ic + 1) * C])
                        else:
                            nc.vector.scalar_tensor_tensor(
                                ot, ac[t][:, 0:C], zr,
                                xth_sb[:, ic * C:(ic + 1) * C],
                                mybir.AluOpType.mult, mybir.AluOpType.add)
                        nc.sync.dma_start(
                            out=out_t[ic * 128:(ic + 1) * 128, :], in_=ot)
    nc.finalize()
    return nc


_CACHE = {}


def _get_nc(n_iter: int = 1):
    if n_iter not in _CACHE:
        _CACHE[n_iter] = build_nc(n_iter)
    return _CACHE[n_iter]


def make_in_maps(x, Wq, bq, Wk, bk, Wv, bv, gamma):
    x = np.asarray(x, dtype=np.float32)
    Wq = np.asarray(Wq, dtype=np.float32)
    bq = np.asarray(bq, dtype=np.float32)
    Wk = np.asarray(Wk, dtype=np.float32)
    bk = np.asarray(bk, dtype=np.float32)
    Wv = np.asarray(Wv, dtype=np.float32)
    bv = np.asarray(bv, dtype=np.float32)
    gamma = np.asarray(gamma, dtype=np.float32).reshape(())

    bf = ml_dtypes.bfloat16
    wqt = np.ascontiguousarray(Wq.T).astype(bf)       # [C, RD]
    wkt = np.ascontiguousarray(Wk.T).astype(bf)       # [C, RD]
    wvt = np.zeros((C, CA), dtype=np.float32)         # [Wv.T | 0 | 0]
    wvt[:, :C] = Wv.T
    wvt = wvt.astype(bf)
    bvz = np.zeros((1, CA), dtype=np.float32)         # [0, 1/gamma, 0]
    with np.errstate(divide="ignore"):
        bvz[0, C] = np.float32(1.0) / gamma           # inf if gamma==0 ->
    one_r = np.ones((1, 128), dtype=np.float32)       # recip(inf)=0 -> out=x
    bq2 = bq.reshape(RD, 1)
    bk2 = bk.reshape(RD, 1)

    in_maps = []
    for c in range(NCORES):
        b, half = divmod(c, 2)
        xb = x[b].reshape(C, N)
        # roll so this core's query half sits at columns 0:NI (the kernel
        # reads queries from xr[:, 0:NI]); k/v use all columns so the roll
        # only permutes j, and the ones-column Z is permutation-invariant
        xbr = np.ascontiguousarray(np.roll(xb, -half * NI, axis=1)).astype(bf)
        xthh = np.ascontiguousarray(
            xb[:, half * NI:(half + 1) * NI].T
            + np.float32(gamma) * bv[None, :])
        in_maps.append({
            "xr": xbr, "xth": xthh,
            "wqt": wqt, "wkt": wkt, "wvt": wvt,
            "bq": bq2, "bk": bk2, "bvz": bvz, "one_r": one_r,
        })
    return in_maps


def assemble(results):
    out = np.empty((B, C, N), dtype=np.float32)
    for c in range(NCORES):
        b, half = divmod(c, 2)
        out[b][:, half * NI:(half + 1) * NI] = results[c]["out"].T
    return out.reshape(B, C, H, W)


def kernel(x, Wq, bq, Wk, bk, Wv, bv, gamma):
    nc = _get_nc(1)
    in_maps = make_in_maps(x, Wq, bq, Wk, bk, Wv, bv, gamma)
    res = run_bass_kernel_spmd(nc, in_maps, list(range(NCORES)))
    return assemble(res.results)
